# revision 28
# baseline (speedup 1.0000x reference)
"""Trainium2 Bass kernel for the 2-layer LSTM language-model problem.

Strategy (8 NeuronCores, SPMD):
  - Tensor-parallel over the 4*NN gate dimension: core k owns hidden chunk k
    (128 of 1024 hidden units) of BOTH LSTM layers; the per-step hidden state
    is re-assembled with an AllGather of transposed h-chunks.
  - Phase A: s0x[t,b,:] = inputs @ (emb_matrix @ W0x_chunk) + b0_chunk
    precomputed for all timesteps (associativity fuses the embedding).
  - Phase B: 200 recurrent steps. Per step and layer: 8..16 accumulating
    matmuls (activations stationary, weights streaming from SBUF), sigmoid/
    tanh on ScalarE, elementwise on DVE, PE transpose of the new h chunk,
    AllGather. Gathered h1T is also scattered (per-core rotated) into a DRAM
    history buffer so phase C can read its row shard at static addresses.
  - Phase C: output MLP on a 25-timestep shard per core:
    z1T = relu(ow0.T @ hsT + b0) computed transposed (weights stationary),
    then logits rows = z1 @ ow1 + b1 (activations stationary).
Host side only reshapes/slices numpy inputs and concatenates the 8 output
row-shards.
"""

import sys
import os

for _p in ("/opt/trn_rl_repo", "/root/.axon_site/_ro/trn_rl_repo"):
    if os.path.isdir(_p) and _p not in sys.path:
        sys.path.insert(0, _p)

import numpy as np
import ml_dtypes

BF = ml_dtypes.bfloat16

import concourse.bass as bass
import concourse.mybir as mybir
import concourse.tile as tile
from concourse import bacc
from concourse.bass_utils import run_bass_kernel_spmd
from concourse.masks import make_identity

F32 = mybir.dt.float32
BF16 = mybir.dt.bfloat16
I32 = mybir.dt.int32
AF = mybir.ActivationFunctionType

# problem shapes (hardcoded per contract)
T, B, V, E, NN, ON = 200, 64, 256, 512, 1024, 1024
N_CORES = 8
CH = NN // N_CORES          # 128 hidden units per core
NG = 4 * CH                 # 512 gate columns per core
KT0 = NN // 128             # 8 k-tiles for h-part contractions
VT = V // 128               # 2 v-tiles
ET = E // 128               # 4 e-tiles
MT = ON // 128              # 8 hid2 tiles

MM_MODE = "f32r"   # "bf16" | "f32r" | "f32"  (phases B/C; phase A stays f32r)

_CACHE = {}


def _build_program(nsteps: int, use_ag: bool = True, variant: str = 'full'):
    """Build the SPMD Bass program (identical for every core)."""
    nblocks = (nsteps * B) // 512          # phase-A row blocks of 512
    ts_shard = nsteps // N_CORES           # phase-C timesteps per core
    rows = ts_shard * B

    SD = {"bf16": BF16, "f32r": mybir.dt.float32r,
          "f32": F32}[MM_MODE]
    SDA = mybir.dt.float32r          # phase-A (embedding/s0x) matmul dtype
    nc = bacc.Bacc("TRN2", target_bir_lowering=False, debug=False,
                   num_devices=N_CORES)

    def _mmc(out, l, r, **kw):
        nc.tensor.matmul(out, l, r, **kw)

    def din(name, shape, dt=F32):
        return nc.dram_tensor(name, shape, dt, kind="ExternalInput").ap()

    inputsT = din("inputsT", [V, nsteps * B], SDA)       # replicated
    emT = din("emT", [E, V], SDA)                        # emb_matrix.T, replicated
    w0x = din("w0x", [E, NG], SDA)                       # lstm_w0[:E, cols_k]
    w0h = din("w0h", [NN, NG], SD)                      # lstm_w0[E:, cols_k]
    w1 = din("w1", [2 * NN, NG], SD)                    # lstm_w1[:, cols_k]
    b0c = din("b0c", [128, NG])                     # b0[cols_k] bcast rows
    b1c = din("b1c", [B, NG], mybir.dt.float32r)                       # b1[cols_k] bcast rows
    h01T_i = din("h01T", [2 * NN, B], BF16)  # interleaved [k,c] h0/h1 init, T
    h1Tc_i = din("h1Tc", [CH, B], BF16)      # own h1 init chunk, transposed
    c0_i = din("c0c", [B, CH])                      # c0[:, chunk_k]
    c1_i = din("c1c", [B, CH])
    ow0 = din("ow0", [NN, ON], SD)                      # out_w0, replicated
    ob0 = din("ob0", [ON, 1])                       # out_b0 column, replicated
    ow1 = din("ow1", [ON, V], SD)                       # out_w1, replicated
    ob1 = din("ob1", [128, V])                      # out_b1 bcast, replicated
    gat = din("gat", [128, nsteps // N_CORES], I32)  # phase-C gather rows

    logits_out = nc.dram_tensor("logits", [rows, V], F32,
                                kind="ExternalOutput").ap()

    with tile.TileContext(nc) as tc:
        with tc.tile_pool(name="dram", bufs=1, space="DRAM") as dram, \
             tc.tile_pool(name="const", bufs=1) as const:
            s0x_d = dram.tile([nsteps * B, NG], mybir.dt.float32r)
            hist_d = dram.tile([nsteps * 128, NG], BF16)
            bounce01 = dram.tile([2 * CH, B], BF16)
            gath01 = nc.dram_tensor("gath01_sh", [2 * NN, B], BF16,
                                    addr_space="Shared").ap()

            ident = const.tile([B, B], F32)
            make_identity(nc, ident[:])
            ident_r = const.tile([B, B], mybir.dt.float32r)
            nc.vector.tensor_copy(ident_r[:], ident[:])

            # ---------------- phase A: s0x precompute ----------------
            with tc.tile_pool(name="pa", bufs=2) as pa, \
                 tc.tile_pool(name="pa_ps", bufs=3, space="PSUM") as pa_ps:
                emT_s = pa.tile([128, ET * V], SDA, tag="emTs")
                nc.sync.dma_start(
                    emT_s[:].rearrange("p (k v) -> p k v", k=ET),
                    emT.rearrange("(k p) v -> p k v", p=128))
                wx_s = pa.tile([128, ET * NG], SDA, tag="wxs")
                nc.sync.dma_start(
                    wx_s[:].rearrange("p (k n) -> p k n", k=ET),
                    w0x.rearrange("(k p) n -> p k n", p=128))
                b0_s = pa.tile([128, NG], F32, tag="b0s")
                nc.sync.dma_start(b0_s[:], b0c[:])

                # W_eff [V, NG] = emb @ W0x_chunk
                we_s = pa.tile([128, VT * NG], SDA, tag="wes")
                for mm in range(VT):
                    ps = pa_ps.tile([128, NG], F32, tag="we_ps")
                    for kk in range(ET):
                        _mmc(
                            ps[:],
                            emT_s[:, kk * V + mm * 128:
                                  kk * V + (mm + 1) * 128],
                            wx_s[:, kk * NG:(kk + 1) * NG],
                            start=(kk == 0), stop=(kk == ET - 1))
                    nc.vector.tensor_copy(
                        we_s[:, mm * NG:(mm + 1) * NG], ps[:])

                for bi in range(nblocks):
                    r0 = bi * 512
                    inT = pa.tile([128, VT * 512], SDA, tag="inT")
                    nc.sync.dma_start(
                        inT[:].rearrange("p (k r) -> p k r", k=VT),
                        inputsT[:, r0:r0 + 512].rearrange(
                            "(k p) r -> p k r", p=128))
                    for mt in range(4):
                        ps = pa_ps.tile([128, NG], F32, tag="sx_ps")
                        for kk in range(VT):
                            _mmc(
                                ps[:],
                                inT[:, kk * 512 + mt * 128:
                                    kk * 512 + (mt + 1) * 128],
                                we_s[:, kk * NG:(kk + 1) * NG],
                                start=(kk == 0), stop=(kk == VT - 1))
                        sx_sb = pa.tile([128, NG], mybir.dt.float32r, tag="sx_sb")
                        nc.vector.tensor_add(sx_sb[:], ps[:], b0_s[:])
                        nc.sync.dma_start(
                            s0x_d[r0 + mt * 128: r0 + (mt + 1) * 128, :],
                            sx_sb[:])

            # ---------------- phase B: recurrence (layer-paired) ----
            # tick t: compute L0(t) and L1(t-1), then ONE AllGather of
            # [h0nT(t) ; h1nT(t-1)] chunks. L1 lags one step; tick nsteps
            # runs only L1(nsteps-1) + final AG for the history store.
            with tc.tile_pool(name="pb_w", bufs=1) as pb_w, \
                 tc.tile_pool(name="pb_g", bufs=2) as pb_g, \
                 tc.tile_pool(name="pb_ps", bufs=2, space="PSUM") as pb_ps, \
                 tc.tile_pool(name="pb_tps", bufs=2, space="PSUM") as pb_tps, \
                 tc.tile_pool(name="pb_sx", bufs=3) as pb_sx, \
                 tc.tile_pool(name="pb_wk", bufs=3) as pb_wk:

                w0h_s = pb_w.tile([128, KT0 * NG], SD)
                nc.sync.dma_start(
                    w0h_s[:].rearrange("p (k n) -> p k n", k=KT0),
                    w0h.rearrange("(k p) n -> p k n", p=128))
                w1_s = pb_w.tile([128, 2 * KT0 * NG], SD)
                nc.sync.dma_start(
                    w1_s[:].rearrange("p (k n) -> p k n", k=2 * KT0),
                    w1.rearrange("(k p) n -> p k n", p=128))
                b1_s = pb_w.tile([B, NG], mybir.dt.float32r)
                nc.sync.dma_start(b1_s[:], b1c[:])
                cst = pb_w.tile([B, 2 * CH], F32)
                nc.sync.dma_start(cst[:, 0:CH], c0_i[:])
                nc.sync.dma_start(cst[:, CH:2 * CH], c1_i[:])
                h1c_s = pb_w.tile([CH, B], BF16)      # own h1 init chunk, T
                nc.sync.dma_start(h1c_s[:], h1Tc_i[:])

                # G layout: slot (kk, c): c=0 -> h0 chunk kk, c=1 -> h1
                # chunk kk, each [128, B]
                def g_slot(G, kk, c):
                    return G[:, (2 * kk + c) * B:(2 * kk + c + 1) * B]

                Gb = pb_g.tile([128, 2 * KT0 * B], BF16, tag="Gb")
                nc.sync.dma_start(
                    Gb[:].rearrange("p (k c b) -> p k c b", k=KT0, c=2),
                    h01T_i.rearrange("(k c p) b -> p k c b", p=128, c=2))
                if SD is BF16:
                    G = Gb             # matmuls read the gather buffer
                else:
                    G = pb_g.tile([128, 2 * KT0 * B], SD, tag="G")
                    nc.vector.tensor_copy(G[:], Gb[:])

                def lstm_chain_parts(gates, c_sl, thc, tag):
                    """Emit (act_fn, dve1_fn, dve2_fn) closures so the two
                    layers' chains can be interleaved; the c-tanh runs
                    jointly for both layers between d1 and d2."""
                    act = pb_wk.tile([B, NG], F32, tag=f"act{tag}")
                    t1 = pb_wk.tile([B, CH], F32, tag=f"t1{tag}")
                    t2 = pb_wk.tile([B, CH], F32, tag=f"t2{tag}")
                    h_new = pb_wk.tile([B, CH], F32, tag=f"h{tag}")

                    def a1():
                        nc.scalar.activation(act[:, 0:3 * CH],
                                             gates[:, 0:3 * CH], AF.Sigmoid)
                        nc.scalar.activation(act[:, 3 * CH:NG],
                                             gates[:, 3 * CH:NG], AF.Tanh)

                    def d1():
                        nc.vector.tensor_mul(t1[:], act[:, 0:CH], c_sl)
                        nc.vector.tensor_mul(t2[:], act[:, CH:2 * CH],
                                             act[:, 3 * CH:NG])
                        nc.vector.tensor_add(c_sl, t1[:], t2[:])

                    def d2():
                        nc.vector.tensor_mul(h_new[:],
                                             act[:, 2 * CH:3 * CH], thc)

                    return h_new, [a1, d1, d2]

                rg = [list(range(N_CORES))]
                nocomm = variant in ("nocomm", "nochain", "mmonly")
                nochain = variant in ("nochain", "mmonly")
                mmonly = variant == "mmonly"
                for t in range(nsteps + 1):
                    do0 = t < nsteps        # L0(t) exists
                    do1 = t > 0             # L1(t-1) exists

                    FR = mybir.dt.float32r
                    if do0:
                        sx_t = pb_sx.tile([B, NG], mybir.dt.float32r, tag="sx")
                        nc.sync.dma_start(sx_t[:],
                                          s0x_d[t * B:(t + 1) * B, :])
                        ps0 = pb_ps.tile([B, NG], F32, tag="ps0")
                        for kk in range(KT0):
                            _mmc(
                                ps0[:], g_slot(G, kk, 0),
                                w0h_s[:, kk * NG:(kk + 1) * NG],
                                start=(kk == 0), stop=False)
                        # fold the precomputed x-part in via an identity
                        # matmul so the activation can read PSUM directly
                        _mmc(ps0[:], ident_r[:], sx_t[:],
                             start=False, stop=True)
                    if do1:
                        ps1 = pb_ps.tile([B, NG], F32, tag="ps1")
                        for kk in range(KT0):
                            _mmc(
                                ps1[:], g_slot(G, kk, 1),
                                w1_s[:, (KT0 + kk) * NG:
                                     (KT0 + kk + 1) * NG],
                                start=(kk == 0), stop=False)
                        for kk in range(KT0):
                            _mmc(
                                ps1[:], g_slot(G, kk, 0),
                                w1_s[:, kk * NG:(kk + 1) * NG],
                                start=False, stop=False)
                        _mmc(ps1[:], ident_r[:], b1_s[:],
                             start=False, stop=True)

                    chain0 = chain1 = None
                    if nochain:
                        if do0:
                            h0n = pb_wk.tile([B, CH], F32, tag="h0")
                            nc.vector.tensor_copy(h0n[:], ps0[:, 0:CH])
                        if do1:
                            h1n = pb_wk.tile([B, CH], F32, tag="h1")
                            nc.vector.tensor_copy(h1n[:], ps1[:, 0:CH])
                    else:
                        thc2 = pb_wk.tile([B, 2 * CH], F32, tag="thc2")
                        if do0:
                            h0n, chain0 = lstm_chain_parts(
                                ps0[:], cst[:, 0:CH], thc2[:, 0:CH], "0")
                        if do1:
                            h1n, chain1 = lstm_chain_parts(
                                ps1[:], cst[:, CH:2 * CH], thc2[:, CH:2 * CH],
                                "1")
                        for stage in range(2):
                            if chain0 is not None:
                                chain0[stage]()
                            if chain1 is not None:
                                chain1[stage]()
                        # joint c-tanh for both layers in one instruction
                        if do0 and do1:
                            nc.scalar.activation(thc2[:], cst[:, 0:2 * CH],
                                                 AF.Tanh)
                        elif do0:
                            nc.scalar.activation(thc2[:, 0:CH],
                                                 cst[:, 0:CH], AF.Tanh)
                        else:
                            nc.scalar.activation(thc2[:, CH:2 * CH],
                                                 cst[:, CH:2 * CH], AF.Tanh)
                        if chain0 is not None:
                            chain0[2]()
                        if chain1 is not None:
                            chain1[2]()
                    if mmonly:
                        continue

                    if nocomm:
                        # transposes only; no bounce/gather/G reload
                        if do0:
                            tps0 = pb_tps.tile([CH, B], F32, tag="tps0")
                            nc.tensor.transpose(tps0[:], h0n[:], ident[:])
                        if do1:
                            tps1 = pb_tps.tile([CH, B], F32, tag="tps1")
                            nc.tensor.transpose(tps1[:], h1n[:], ident[:])
                        continue
                    tsb = pb_wk.tile([128, 2 * B], BF16, tag="tsb")
                    if do0:
                        tps0 = pb_tps.tile([CH, B], F32, tag="tps0")
                        nc.tensor.transpose(tps0[:], h0n[:], ident[:])
                        nc.vector.tensor_copy(tsb[:, 0:B], tps0[:])
                    else:
                        nc.vector.tensor_copy(tsb[:, 0:B], h1c_s[:])
                    if do1:
                        tps1 = pb_tps.tile([CH, B], F32, tag="tps1")
                        nc.tensor.transpose(tps1[:], h1n[:], ident[:])
                        nc.vector.tensor_copy(tsb[:, B:2 * B], tps1[:])
                    elif t == 0:
                        nc.vector.tensor_copy(tsb[:, B:2 * B], h1c_s[:])

                    nc.sync.dma_start(
                        bounce01.rearrange("(c p) b -> p c b", p=128),
                        tsb[:].rearrange("p (c b) -> p c b", c=2))
                    if use_ag:
                        nc.gpsimd.collective_compute(
                            "AllGather", mybir.AluOpType.bypass,
                            replica_groups=rg,
                            ins=[bounce01.opt()], outs=[gath01.opt()])
                    else:
                        nc.sync.dma_start(gath01[0:2 * CH, :],
                                          bounce01[:])
                    Gb = pb_g.tile([128, 2 * KT0 * B], BF16, tag="Gb")
                    gv = Gb[:].rearrange("p (k c b) -> p k c b", k=KT0, c=2)
                    gs = gath01.rearrange("(k c p) b -> p k c b",
                                          p=128, c=2)
                    if SD is BF16:
                        G = Gb
                        for q in range(4):
                            nc.sync.dma_start(gv[:, 2 * q:2 * (q + 1)],
                                              gs[:, 2 * q:2 * (q + 1)])
                    else:
                        G = pb_g.tile([128, 2 * KT0 * B], SD, tag="G")
                        qw = 2 * KT0 * B // 4
                        for q in range(4):
                            nc.sync.dma_start(gv[:, 2 * q:2 * (q + 1)],
                                              gs[:, 2 * q:2 * (q + 1)])
                            nc.vector.tensor_copy(
                                G[:, q * qw:(q + 1) * qw],
                                Gb[:, q * qw:(q + 1) * qw])

                    if do1:
                        # store gathered h1T(t-1) directly into history
                        nc.sync.dma_start(
                            hist_d[(t - 1) * 128:t * 128, :].rearrange(
                                "p (k b) -> p k b", k=KT0),
                            Gb[:].rearrange("p (k c b) -> p k c b",
                                            k=KT0, c=2)[:, :, 1, :])

            # ---------------- phase C: output MLP on row shard ----------
            with tc.tile_pool(name="pc", bufs=2) as pc, \
                 tc.tile_pool(name="pc_ps", bufs=3, space="PSUM") as pc_ps, \
                 tc.tile_pool(name="pc_z", bufs=1) as pc_z:
                gat_s = pc_z.tile([128, ts_shard], I32)
                nc.sync.dma_start(gat_s[:], gat[:])
                hsTb = pc_z.tile([128, ts_shard * NG], BF16)
                for j in range(ts_shard):
                    nc.gpsimd.indirect_dma_start(
                        out=hsTb[:, j * NG:(j + 1) * NG],
                        out_offset=None,
                        in_=hist_d[:],
                        in_offset=bass.IndirectOffsetOnAxis(
                            ap=gat_s[:, j:j + 1], axis=0))
                if SD is BF16:
                    hsT = hsTb         # slot j: h1T(t_j)
                else:
                    hsT = pc_z.tile([128, ts_shard * NG], SD)
                    nc.vector.tensor_copy(hsT[:], hsTb[:])
                ow0_s = pc_z.tile([128, KT0 * ON], SD)
                nc.sync.dma_start(
                    ow0_s[:].rearrange("p (k n) -> p k n", k=KT0),
                    ow0.rearrange("(k p) n -> p k n", p=128))
                ob0_s = pc_z.tile([128, MT], F32)
                nc.sync.dma_start(
                    ob0_s[:].rearrange("p (m o) -> p m o", o=1),
                    ob0.rearrange("(m p) o -> p m o", p=128))
                ow1_s = pc_z.tile([128, MT * V], SD)
                nc.sync.dma_start(
                    ow1_s[:].rearrange("p (k n) -> p k n", k=MT),
                    ow1.rearrange("(k p) n -> p k n", p=128))
                ob1_s = pc_z.tile([128, V], F32)
                nc.sync.dma_start(ob1_s[:], ob1[:])

                z1 = pc_z.tile([128, MT * rows], SD)       # z1T slots
                hsT_v = hsT[:].rearrange("p (j n) -> p j n", j=ts_shard)
                rgs = []
                j0 = 0
                while j0 < ts_shard:                        # 8-step groups
                    jn = min(8, ts_shard - j0)
                    rgs.append((j0, jn))
                    j0 += jn
                for m in range(MT):
                    for (j0, jn) in rgs:
                        ps = pc_ps.tile([128, 512], F32, tag="z_ps")
                        psv = ps[:, 0:jn * B].rearrange(
                            "q (j b) -> q j b", j=jn)
                        for kk in range(KT0):
                            _mmc(
                                psv,
                                ow0_s[:, kk * ON + m * 128:
                                      kk * ON + (m + 1) * 128],
                                hsT_v[:, j0:j0 + jn,
                                      kk * B:(kk + 1) * B],
                                start=(kk == 0), stop=(kk == KT0 - 1))
                        nc.scalar.activation(
                            z1[:, m * rows + j0 * B:
                               m * rows + (j0 + jn) * B],
                            ps[:, 0:jn * B], AF.Relu,
                            bias=ob0_s[:, m:m + 1])

                rt0 = 0
                while rt0 < rows:
                    rn = min(128, rows - rt0)
                    ps = pc_ps.tile([128, V], F32, tag="lg_ps")
                    for m in range(MT):
                        _mmc(
                            ps[0:rn, :],
                            z1[:, m * rows + rt0: m * rows + rt0 + rn],
                            ow1_s[:, m * V:(m + 1) * V],
                            start=(m == 0), stop=(m == MT - 1))
                    lg = pc.tile([128, V], F32, tag="lg_sb")
                    nc.vector.tensor_add(lg[0:rn, :], ps[0:rn, :],
                                         ob1_s[0:rn, :])
                    nc.sync.dma_start(logits_out[rt0:rt0 + rn, :],
                                      lg[0:rn, :])
                    rt0 += rn

    nc.compile()
    return nc


def _prep_in_maps(inputs, nsteps):
    CV = BF if MM_MODE == "bf16" else np.float32
    """Slice/transpose numpy inputs into per-core input maps."""
    x = np.ascontiguousarray(inputs["inputs"], dtype=np.float32)
    x = x.reshape(nsteps * B, V)
    inputsT = np.ascontiguousarray(x.T.astype(np.float32))    # [V, T*B]
    emT = np.ascontiguousarray(
        np.asarray(inputs["emb_matrix"], dtype=np.float32).T)
    w0 = np.asarray(inputs["lstm_w0"], dtype=np.float32)
    w1 = np.asarray(inputs["lstm_w1"], dtype=np.float32)
    b0 = np.asarray(inputs["lstm_b0"], dtype=np.float32)
    b1 = np.asarray(inputs["lstm_b1"], dtype=np.float32)
    h0 = np.asarray(inputs["h0"], dtype=np.float32)
    c0 = np.asarray(inputs["c0"], dtype=np.float32)
    h1 = np.asarray(inputs["h1"], dtype=np.float32)
    c1 = np.asarray(inputs["c1"], dtype=np.float32)
    ow0 = np.ascontiguousarray(
        np.asarray(inputs["out_w0"], dtype=np.float32).astype(CV))
    ob0 = np.ascontiguousarray(
        np.asarray(inputs["out_b0"], dtype=np.float32).reshape(ON, 1))
    ow1 = np.ascontiguousarray(
        np.asarray(inputs["out_w1"], dtype=np.float32).astype(CV))
    ob1 = np.ascontiguousarray(
        np.broadcast_to(inputs["out_b1"], (128, V)), dtype=np.float32)
    # interleaved init state: rows (2k+c)*128.. = (h0 if c==0 else h1) chunk k
    h01T = np.empty((2 * NN, B), BF)
    h0T_, h1T_ = h0.T, h1.T
    for kk in range(KT0):
        h01T[(2 * kk) * 128:(2 * kk + 1) * 128] = h0T_[kk * 128:(kk + 1) * 128]
        h01T[(2 * kk + 1) * 128:(2 * kk + 2) * 128] = \
            h1T_[kk * 128:(kk + 1) * 128]

    ts_shard = nsteps // N_CORES
    in_maps = []
    for k in range(N_CORES):
        cols = np.concatenate([
            np.arange(g * NN + k * CH, g * NN + (k + 1) * CH)
            for g in range(4)])
        S = ts_shard * k
        p = np.arange(128)
        gat_rows = np.ascontiguousarray(
            ((S + np.arange(ts_shard))[None, :] * 128
             + p[:, None]).astype(np.int32))
        in_maps.append({
            "inputsT": inputsT,
            "emT": emT,
            "w0x": np.ascontiguousarray(w0[:E, cols].astype(np.float32)),
            "w0h": np.ascontiguousarray(w0[E:, cols].astype(CV)),
            "w1": np.ascontiguousarray(w1[:, cols].astype(CV)),
            "b0c": np.ascontiguousarray(
                np.broadcast_to(b0[cols], (128, NG))),
            "b1c": np.ascontiguousarray(
                np.broadcast_to(b1[cols], (B, NG))),
            "h01T": h01T,
            "h1Tc": np.ascontiguousarray(
                h1.T[k * CH:(k + 1) * CH, :].astype(BF)),
            "c0c": np.ascontiguousarray(c0[:, k * CH:(k + 1) * CH]),
            "c1c": np.ascontiguousarray(c1[:, k * CH:(k + 1) * CH]),
            "ow0": ow0,
            "ob0": ob0,
            "ow1": ow1,
            "ob1": ob1,
            "gat": gat_rows,
        })
    return in_maps


def kernel(**inputs):
    nsteps = inputs["inputs"].shape[0]
    if nsteps not in _CACHE:
        _CACHE[nsteps] = _build_program(nsteps)
    nc = _CACHE[nsteps]
    in_maps = _prep_in_maps(inputs, nsteps)
    res = run_bass_kernel_spmd(nc, in_maps, list(range(N_CORES)))
    logits = np.concatenate(
        [res.results[k]["logits"] for k in range(N_CORES)], axis=0)
    return logits

